# revision 23
# baseline (speedup 1.0000x reference)
"""Trainium2 Bass kernel for nn_AttentionADCell.

Computation (reference):
  h   = LayerNorm(relu(x @ Wh + bh)) * ln_scale + ln_bias          (B, DH)
  xc  = [repeat(x, A), one_hot(A)]                                  (B*A, DIN+A)
  z1  = (xc @ Wz1).reshape(-1, H, DH) ; z2 likewise
  w   = einsum('bhg,bhd->bgd', z2, z1)  -> rownorm over d -> tanh
  y_  = einsum('bgd,bd->bg', w, repeat(h, A))                       (B*A, DH)
  q   = mean(y_**2, axis=1).reshape(B, A)                           (B, A)
returns (h, q, y_)

Structure exploited:
  z1[b*A+a] = u1[b] + c1[a]  where u1 = x @ Wz1[:DIN], c1[a] = Wz1[DIN+a]
  row-norm mean/std of w computed analytically from per-row Gram matrices of
  the staged z1 rows, folded into the bilinear operands (z2' = z2*inv_sigma,
  augmented row: z1_aug = -1, z2'_aug = mu*inv_sigma) so PSUM holds the
  normalized pre-tanh values directly.

Sharding: data-parallel over batch. 8 cores x 8 batches (=128 rows) each.
Staging: 4 rows per [128, 512] bf16 tile, one row per 32-partition strip
(compute-engine APs must start at partition 0/32/64/96).
"""

import os
from contextlib import ExitStack
import numpy as np
import ml_dtypes

import concourse.bass as bass
import concourse.tile as tile
from concourse import bacc, mybir
from concourse.bass_utils import run_bass_kernel_spmd

# ---------------- problem constants (hardcoded) ----------------
B, DIN, DH, A, H = 64, 512, 512, 16, 8
G = DH
N_CORES = 8
NB = int(os.environ.get("K_NB", str(B // N_CORES)))  # batches per core (8)
R = NB * A                 # rows per core (128)
EPS = 1e-6
KAUG = H + 1               # 9 contraction rows (8 heads + aug)
RPT = 4                    # rows per staging tile (one per 32-strip)
N_TILES = R // RPT
NCH = DIN // 128
F32 = mybir.dt.float32
BF16 = mybir.dt.bfloat16
AX = mybir.AluOpType
AF = mybir.ActivationFunctionType
OBW = 124 + 128            # wide-mask width for accumulate-stats matmuls
K_STOP = int(os.environ.get("K_STOP", "9"))

_cache = {}


def _ap_with(ap2d, part_stride_mult, count):
    dims = [list(d) for d in ap2d.ap]
    dims[0] = [dims[0][0] * part_stride_mult, count]
    return bass.AP(tensor=ap2d.tensor, offset=ap2d.offset, ap=dims)


def _bcast(ap_row, count):
    """Partition-broadcast a [1, ...] (DRAM) AP to `count` partitions."""
    dims = [list(d) for d in ap_row.ap]
    assert dims[0][1] == 1, dims
    dims[0] = [0, count]
    return bass.AP(tensor=ap_row.tensor, offset=ap_row.offset, ap=dims)


def host_constants():
    bf = ml_dtypes.bfloat16
    identb = np.eye(128, dtype=np.float32).astype(bf)
    # accumulate masks (bf16): lhsT slice [*, 124-4m : 252-4m] puts row j of
    # tile m (strip j) at output partition 4m+j.
    obw = np.zeros((128, OBW), dtype=np.float32)   # E: sum heads k<8
    pmw = np.zeros((128, OBW), dtype=np.float32)   # Mu: pick aug row k=8
    for p in range(128):
        s, k = p // 32, p % 32
        if k < 8:
            obw[p, 124 + s] = 1.0
        elif k == 8:
            pmw[p, 124 + s] = 1.0
    # block-diag gram mask: keep gsta[p=(s,k), q=(s',m)] iff s==s' and k<8
    bdm = np.zeros((128, 128), dtype=np.float32)
    for p in range(128):
        s, k = p // 32, p % 32
        if k < 8:
            bdm[p, 32 * s:32 * s + 32] = 1.0
    ones_row = np.ones((1, DH), dtype=np.float32).astype(bf)
    return identb, obw.astype(bf), pmw.astype(bf), bdm.astype(bf), ones_row


def build_program():
    if "nc" in _cache:
        return _cache["nc"]
    nc = bacc.Bacc("TRN2", target_bir_lowering=False, debug=False)

    # ---- DRAM I/O ----
    d_xT = nc.dram_tensor("xT", (DIN, NB), F32, kind="ExternalInput")
    d_Wh = nc.dram_tensor("Wh", (DIN, DH), F32, kind="ExternalInput")
    d_bh = nc.dram_tensor("bh", (DH,), F32, kind="ExternalInput")
    d_lns = nc.dram_tensor("ln_scale", (DH,), F32, kind="ExternalInput")
    d_lnb = nc.dram_tensor("ln_bias", (DH,), F32, kind="ExternalInput")
    d_Wz1 = nc.dram_tensor("Wz1", (DIN + A, H * DH), F32, kind="ExternalInput")
    d_Wz2 = nc.dram_tensor("Wz2", (DIN + A, H * DH), F32, kind="ExternalInput")
    d_id = nc.dram_tensor("identb", (128, 128), BF16, kind="ExternalInput")
    d_ob = nc.dram_tensor("obw", (128, OBW), BF16, kind="ExternalInput")
    d_pm = nc.dram_tensor("pmw", (128, OBW), BF16, kind="ExternalInput")
    d_bdm = nc.dram_tensor("bdm", (128, 128), BF16, kind="ExternalInput")
    d_onesr = nc.dram_tensor("ones_row", (1, DH), BF16, kind="ExternalInput")

    d_h = nc.dram_tensor("h_out", (NB, DH), F32, kind="ExternalOutput")
    d_vbf = nc.dram_tensor("vbf_scratch", (NB, DH), BF16)
    d_u1 = nc.dram_tensor("u1_scratch", (NB, H * DH), BF16)
    d_u2 = nc.dram_tensor("u2_scratch", (NB, H * DH), BF16)
    d_invsig = nc.dram_tensor("invsig_scratch", (R, G), BF16)
    d_y = nc.dram_tensor("y_raw", (128, 4 * R), F32, kind="ExternalOutput")

    def _body(tc, ctx):
        consts = ctx.enter_context(tc.tile_pool(name="consts", bufs=1))
        persist = ctx.enter_context(tc.tile_pool(name="persist", bufs=1))
        stg = ctx.enter_context(tc.tile_pool(name="staging", bufs=1))
        wstream = ctx.enter_context(tc.tile_pool(name="wstream", bufs=4))
        statss = ctx.enter_context(tc.tile_pool(name="statss", bufs=2))

        # ---------- constants ----------
        identb = consts.tile([128, 128], BF16)
        nc.sync.dma_start(identb[:], d_id[:])
        obw = consts.tile([128, OBW], BF16)
        nc.sync.dma_start(obw[:], d_ob[:])
        pmw = consts.tile([128, OBW], BF16)
        nc.sync.dma_start(pmw[:], d_pm[:])
        bdm = consts.tile([128, 128], BF16)
        nc.sync.dma_start(bdm[:], d_bdm[:])
        ones_col = consts.tile([128, 1], F32)
        nc.vector.memset(ones_col[:], 1.0)
        eps_col = consts.tile([NB, 1], F32)
        nc.vector.memset(eps_col[:], EPS)

        xTs = consts.tile([128, NCH * NB], F32)
        for i in range(NCH):
            nc.sync.dma_start(xTs[:, i * NB:(i + 1) * NB],
                              d_xT[128 * i:128 * (i + 1), :])
        xTb = consts.tile([128, NCH * NB], BF16)
        nc.vector.tensor_copy(xTb[:], xTs[:])
        whs = []
        for i in range(NCH):
            t = consts.tile([128, DH], F32, name=f"whs_{i}")
            nc.sync.dma_start(t[:], d_Wh[128 * i:128 * (i + 1), :])
            whs.append(t)
        bh_r = consts.tile([NB, DH], F32)
        nc.sync.dma_start(bh_r[:],
                          _bcast(d_bh[:].rearrange("(o d) -> o d", o=1), NB))
        lns_r = consts.tile([NB, DH], F32)
        nc.sync.dma_start(lns_r[:],
                          _bcast(d_lns[:].rearrange("(o d) -> o d", o=1), NB))
        lnb_r = consts.tile([NB, DH], F32)
        nc.sync.dma_start(lnb_r[:],
                          _bcast(d_lnb[:].rearrange("(o d) -> o d", o=1), NB))

        # c tail rows -> bf16 [8a+h, d], then strip-gathered c tiles
        C1f = persist.tile([128, DH], F32)
        C2f = persist.tile([128, DH], F32)
        for a in range(A):
            nc.sync.dma_start(C1f[8 * a:8 * a + 8, :],
                              d_Wz1[DIN + a, :].rearrange("(h d) -> h d", h=H))
            nc.sync.dma_start(C2f[8 * a:8 * a + 8, :],
                              d_Wz2[DIN + a, :].rearrange("(h d) -> h d", h=H))
        C1b = persist.tile([128, DH], BF16)
        nc.vector.tensor_copy(C1b[:], C1f[:])
        C2b = persist.tile([128, DH], BF16)
        nc.vector.tensor_copy(C2b[:], C2f[:])
        NCB = max(1, A // RPT)
        c1gP, c2gP = [], []
        for mb in range(NCB):
            t1 = persist.tile([128, DH], BF16, name=f"c1gP_{mb}")
            t2 = persist.tile([128, DH], BF16, name=f"c2gP_{mb}")
            nc.gpsimd.memset(t1[:], 0.0)
            nc.gpsimd.memset(t2[:], 0.0)
            for j in range(RPT):
                a = RPT * mb + j
                nc.gpsimd.dma_start(t1[32 * j:32 * j + 8, :],
                                    C1b[8 * a:8 * a + 8, :])
                nc.gpsimd.dma_start(t2[32 * j:32 * j + 8, :],
                                    C2b[8 * a:8 * a + 8, :])
                # Z2 aug row carries 1.0 so prod = Z2*zG exposes mu via pmw
                nc.gpsimd.dma_start(t2[32 * j + 8:32 * j + 9, :],
                                    d_onesr[:])
            c1gP.append(t1)
            c2gP.append(t2)

        pstats_cm = tc.tile_pool(name="pstats", bufs=2, space="PSUM")
        pstats = pstats_cm.__enter__()

        # ---------- h path ----------
        ph = pstats.tile([NB, DH], F32, tag="u")
        for i in range(NCH):
            nc.tensor.matmul(ph[:], xTs[:, i * NB:(i + 1) * NB], whs[i][:],
                             start=(i == 0), stop=(i == NCH - 1))
        hpre = persist.tile([NB, DH], F32)
        nc.vector.tensor_add(hpre[:], ph[:], bh_r[:])
        nc.scalar.activation(hpre[:], hpre[:], AF.Relu)
        hstats = persist.tile([NB, 6], F32)
        nc.vector.bn_stats(hstats[:], hpre[:])
        hmv = persist.tile([NB, 2], F32)
        nc.vector.bn_aggr(hmv[:], hstats[:])
        hstd = persist.tile([NB, 1], F32)
        nc.scalar.activation(hstd[:], hmv[:, 1:2], AF.Sqrt, bias=eps_col[:])
        hrstd = persist.tile([NB, 1], F32)
        nc.vector.reciprocal(hrstd[:], hstd[:])
        h_s = persist.tile([NB, DH], F32)
        nc.vector.tensor_scalar(out=h_s[:], in0=hpre[:], scalar1=hmv[:, 0:1],
                                scalar2=hrstd[:], op0=AX.subtract, op1=AX.mult)
        nc.vector.tensor_mul(h_s[:], h_s[:], lns_r[:])
        nc.vector.tensor_add(h_s[:], h_s[:], lnb_r[:])
        nc.sync.dma_start(d_h[:], h_s[:])
        v_bf = persist.tile([NB, DH], BF16)
        nc.vector.tensor_copy(v_bf[:], h_s[:])
        nc.sync.dma_start(d_vbf[:], v_bf[:])
        vrep = persist.tile([128, NB * DH], BF16)
        for b in range(NB):
            nc.sync.dma_start(vrep[:, DH * b:DH * (b + 1)],
                              _bcast(d_vbf[b:b + 1, :], 128))

        if K_STOP < 2:
            return
        # ---------- u1/u2 (bf16 matmuls, wide evac, DRAM roundtrip) --------
        Uw1 = persist.tile([NB, H * DH], BF16)
        Uw2 = persist.tile([NB, H * DH], BF16)
        for dW, Uw, dU in ((d_Wz1, Uw1, d_u1), (d_Wz2, Uw2, d_u2)):
            for hh in range(H):
                pu = pstats.tile([NB, DH], F32, tag="u", name=f"pu_{hh}")
                for i in range(NCH):
                    wz = wstream.tile([128, DH], F32, tag="wz", name="wz")
                    dma_e = nc.sync if (hh + i) % 2 == 0 else nc.gpsimd
                    dma_e.dma_start(
                        wz[:], dW[128 * i:128 * (i + 1), DH * hh:DH * (hh + 1)])
                    nc.tensor.matmul(pu[:], xTs[:, i * NB:(i + 1) * NB], wz[:],
                                     start=(i == 0), stop=(i == NCH - 1))
                nc.vector.tensor_copy(Uw[:, DH * hh:DH * (hh + 1)], pu[:])
            nc.sync.dma_start(dU[:], Uw[:])

        if K_STOP < 3:
            return
        # ---------- per-batch u staging + per-tile Z = Zu + c --------------
        Zu1 = [persist.tile([128, DH], BF16, name=f"Zu1_{b}")
               for b in range(NB)]
        Zu2 = [persist.tile([128, DH], BF16, name=f"Zu2_{b}")
               for b in range(NB)]
        for b in range(NB):
            nc.gpsimd.memset(Zu1[b][:], -1.0)
            nc.gpsimd.memset(Zu2[b][:], 0.0)
            for j in range(RPT):
                nc.sync.dma_start(
                    Zu1[b][32 * j:32 * j + 8, :],
                    d_u1[b, :].rearrange("(h d) -> h d", h=H))
                nc.sync.dma_start(
                    Zu2[b][32 * j:32 * j + 8, :],
                    d_u2[b, :].rearrange("(h d) -> h d", h=H))
        Z1 = [stg.tile([128, DH], BF16, tag=f"z1_{m}", name=f"Z1_{m}")
              for m in range(N_TILES)]
        Z2 = [stg.tile([128, DH], BF16, tag=f"z2_{m}", name=f"Z2_{m}")
              for m in range(N_TILES)]
        for m in range(N_TILES):
            bl = (RPT * m) // A
            nc.gpsimd.tensor_add(Z1[m][:], Zu1[bl][:], c1gP[m % NCB][:])
            nc.vector.tensor_add(Z2[m][:], Zu2[bl][:], c2gP[m % NCB][:])
        # transposes of Zu1 / c1gP once; per-tile z1T = sum (linearity)
        Zu1T = []
        c1gPT = []
        for b in range(NB):
            pt = pstats.tile([128, 512], BF16, tag="pt", name=f"ptu_{b}",
                             bufs=1)
            for c in range(4):
                nc.tensor.transpose(pt[:, 128 * c:128 * (c + 1)],
                                    Zu1[b][:, 128 * c:128 * (c + 1)],
                                    identb[:])
            t = persist.tile([128, 512], BF16, name=f"Zu1T_{b}")
            nc.vector.tensor_copy(t[:], pt[:])
            Zu1T.append(t)
        for mb in range(NCB):
            pt = pstats.tile([128, 512], BF16, tag="pt", name=f"ptc_{mb}",
                             bufs=1)
            for c in range(4):
                nc.tensor.transpose(pt[:, 128 * c:128 * (c + 1)],
                                    c1gP[mb][:, 128 * c:128 * (c + 1)],
                                    identb[:])
            t = persist.tile([128, 512], BF16, name=f"c1gPT_{mb}")
            nc.vector.tensor_copy(t[:], pt[:])
            c1gPT.append(t)

        if K_STOP < 4:
            return
        # ---------- per-tile stats ----------
        pE = pstats.tile([128, G], F32, tag="pE", bufs=1)
        pMu = pstats.tile([128, G], F32, tag="pMu", bufs=1)
        for m in range(N_TILES):
            bl = (RPT * m) // A
            z1T = statss.tile([128, 512], BF16, tag="z1T", name=f"z1T_{m}")
            nc.vector.tensor_add(z1T[:], Zu1T[bl][:], c1gPT[m % NCB][:])
            # merged gram: full [128,128] cross-gram, 4 k-chunks
            pG = pstats.tile([128, 128], F32, tag="pG", name=f"pG_{m}",
                             bufs=1)
            for c in range(4):
                sl = slice(128 * c, 128 * (c + 1))
                nc.tensor.matmul(pG[:], z1T[:, sl], z1T[:, sl],
                                 start=(c == 0), stop=(c == 3))
            gsta = statss.tile([128, 128], BF16, tag="gsta", name=f"gsta_{m}")
            nc.vector.tensor_mul(gsta[:], pG[:], bdm[:])
            # merged zG: one matmul (gsta block-masked, Z2 pads zero)
            pzG = pstats.tile([128, G], F32, tag="pzG", name=f"pzG_{m}")
            nc.tensor.matmul(pzG[:], gsta[:], Z2[m][:], start=True, stop=True)
            prod = statss.tile([128, G], BF16, tag="prod", name=f"prod_{m}")
            nc.vector.tensor_mul(prod[:], Z2[m][:], pzG[:])
            o = 124 - 4 * m
            nc.tensor.matmul(pE[:], obw[:, o:o + 128], prod[:],
                             start=(m == 0), stop=(m == N_TILES - 1),
                             skip_group_check=True)
            nc.tensor.matmul(pMu[:], pmw[:, o:o + 128], prod[:],
                             start=(m == 0), stop=(m == N_TILES - 1),
                             skip_group_check=True)

        if K_STOP < 5:
            return
        # ---------- finalize inv_sigma ----------
        EAll = persist.tile([128, G], F32)
        nc.vector.tensor_scalar_mul(EAll[:], pE[:], 1.0 / G)
        MuAll = persist.tile([128, G], F32)
        nc.vector.tensor_scalar_mul(MuAll[:], pMu[:], -1.0 / G)
        var = persist.tile([128, G], F32)
        nc.vector.tensor_mul(var[:], MuAll[:], MuAll[:])
        nc.vector.tensor_sub(var[:], EAll[:], var[:])
        std = persist.tile([128, G], F32)
        nc.scalar.activation(std[:], var[:], AF.Sqrt)
        nc.vector.tensor_scalar_add(std[:], std[:], EPS)
        invsig = persist.tile([128, G], F32)
        nc.vector.reciprocal(invsig[:], std[:])
        invsigB = persist.tile([128, G], BF16)
        nc.vector.tensor_copy(invsigB[:], invsig[:])
        nc.sync.dma_start(d_invsig[:], invsigB[0:R, :])
        MuBf = persist.tile([128, G], BF16)
        nc.vector.tensor_copy(MuBf[:], MuAll[:])

        if K_STOP < 6:
            return
        # ---------- scale staging: Z2p = (Z2 + mu at aug) * invsig ---------
        ireps = []
        for i in range(2):
            t = persist.tile([128, G], BF16, name=f"irep_{i}")
            nc.gpsimd.memset(t[:], 0.0)
            ireps.append(t)
        qeng = [nc.sync, nc.gpsimd, nc.sync, nc.gpsimd]
        for m in range(N_TILES):
            irep = ireps[m % 2]
            # merged aug-row mu write: partitions {8,40,72,104}
            nc.sync.dma_start(_ap_with(Z2[m][8:9, :], 32, 4),
                                MuBf[RPT * m:RPT * m + RPT, :])
            for j in range(RPT):
                r = RPT * m + j
                qeng[j].dma_start(irep[32 * j:32 * j + 9, :],
                                  _bcast(d_invsig[r:r + 1, :], 9))
            nc.vector.tensor_mul(Z2[m][:], Z2[m][:], irep[:])

        pstats_cm.__exit__(None, None, None)
        if K_STOP < 7:
            return

        # ---------- main: bilinear + tanh + y-contraction ----------
        pmain_cm = tc.tile_pool(name="pmain", bufs=2, space="PSUM")
        pmain = pmain_cm.__enter__()
        Yall = persist.tile([128, 4 * R], F32)
        for m in range(N_TILES):
            for j in range(RPT):
                r = RPT * m + j
                bl = r // A
                p0 = 32 * j
                pw = pmain.tile([128, 4 * G], F32, tag="pw", name=f"pw_{r}")
                for c in range(4):
                    nc.tensor.matmul(pw[:, G * c:G * (c + 1)],
                                     Z2[m][p0:p0 + KAUG, 128 * c:128 * (c + 1)],
                                     Z1[m][p0:p0 + KAUG, :],
                                     start=True, stop=True,
                                     tile_position=(p0, 0))
                ts_ = statss.tile([128, 4 * G], BF16, tag="tanh", name=f"t_{r}")
                nc.scalar.activation(ts_[:], pw[:], AF.Tanh)
                if K_STOP < 8:
                    continue
                tdump = statss.tile([128, G], BF16, tag="tdump", name=f"td_{r}")
                for c in range(4):
                    nc.vector.scalar_tensor_tensor(
                        out=tdump[:], in0=ts_[:, G * c:G * (c + 1)],
                        scalar=1.0, in1=vrep[:, DH * bl:DH * (bl + 1)],
                        op0=AX.mult, op1=AX.mult,
                        accum_out=Yall[:, 4 * r + c:4 * r + c + 1])
        if K_STOP < 9:
            return
        nc.sync.dma_start(d_y[:], Yall[:])
        pmain_cm.__exit__(None, None, None)

    with tile.TileContext(nc) as tc, ExitStack() as ctx:
        _body(tc, ctx)
    nc.compile()
    _cache["nc"] = nc
    return nc


def kernel(x, Wh, bh, ln_scale, ln_bias, Wz1, Wz2):
    x = np.ascontiguousarray(np.asarray(x, dtype=np.float32))
    Wh = np.ascontiguousarray(np.asarray(Wh, dtype=np.float32))
    bh = np.ascontiguousarray(np.asarray(bh, dtype=np.float32))
    ln_scale = np.ascontiguousarray(np.asarray(ln_scale, dtype=np.float32))
    ln_bias = np.ascontiguousarray(np.asarray(ln_bias, dtype=np.float32))
    Wz1 = np.ascontiguousarray(np.asarray(Wz1, dtype=np.float32))
    Wz2 = np.ascontiguousarray(np.asarray(Wz2, dtype=np.float32))

    nc = build_program()
    identb, obw, pmw, bdm, ones_row = host_constants()
    in_maps = []
    for c in range(N_CORES):
        xT = np.ascontiguousarray(x[NB * c:NB * (c + 1), :].T)
        in_maps.append({
            "xT": xT, "Wh": Wh, "bh": bh, "ln_scale": ln_scale,
            "ln_bias": ln_bias, "Wz1": Wz1, "Wz2": Wz2,
            "identb": identb, "obw": obw, "pmw": pmw, "bdm": bdm,
            "ones_row": ones_row,
        })
    res = run_bass_kernel_spmd(nc, in_maps, core_ids=list(range(N_CORES)),
                               trace=bool(int(os.environ.get("K_TRACE", "0"))))
    if res.exec_time_ns is not None:
        _cache["exec_time_ns"] = res.exec_time_ns
        _cache["trace"] = res.instructions_and_trace
    h = np.concatenate([r["h_out"] for r in res.results], axis=0)
    ys = []
    for r_ in res.results:
        yr = r_["y_raw"].reshape(128, R, 4)       # p, r, c
        ys.append(np.transpose(yr, (1, 2, 0)).reshape(R, G))
    y_ = np.concatenate(ys, axis=0)
    q = (y_.astype(np.float64) ** 2).mean(axis=1).astype(np.float32)
    q = q.reshape(B, A) if NB * N_CORES == B else q.reshape(-1, A)
    return h, q, y_


# revision 28
# speedup vs baseline: 1.0298x; 1.0298x over previous
"""Trainium2 Bass kernel for nn_AttentionADCell.

Computation (reference):
  h   = LayerNorm(relu(x @ Wh + bh)) * ln_scale + ln_bias          (B, DH)
  xc  = [repeat(x, A), one_hot(A)]                                  (B*A, DIN+A)
  z1  = (xc @ Wz1).reshape(-1, H, DH) ; z2 likewise
  w   = einsum('bhg,bhd->bgd', z2, z1)  -> rownorm over d -> tanh
  y_  = einsum('bgd,bd->bg', w, repeat(h, A))                       (B*A, DH)
  q   = mean(y_**2, axis=1).reshape(B, A)                           (B, A)
returns (h, q, y_)

Structure exploited:
  z1[b*A+a] = u1[b] + c1[a]  where u1 = x @ Wz1[:DIN], c1[a] = Wz1[DIN+a]
  row-norm mean/std of w computed analytically from per-row Gram matrices of
  the staged z1 rows, folded into the bilinear operands (z2' = z2*inv_sigma,
  augmented row: z1_aug = -1, z2'_aug = mu*inv_sigma) so PSUM holds the
  normalized pre-tanh values directly.

Sharding: data-parallel over batch. 8 cores x 8 batches (=128 rows) each.
Staging: 4 rows per [128, 512] bf16 tile, one row per 32-partition strip
(compute-engine APs must start at partition 0/32/64/96).
"""

import os
from contextlib import ExitStack
import numpy as np
import ml_dtypes

import concourse.bass as bass
import concourse.tile as tile
from concourse import bacc, mybir
from concourse.bass_utils import run_bass_kernel_spmd

# ---------------- problem constants (hardcoded) ----------------
B, DIN, DH, A, H = 64, 512, 512, 16, 8
G = DH
N_CORES = 8
NB = int(os.environ.get("K_NB", str(B // N_CORES)))  # batches per core (8)
R = NB * A                 # rows per core (128)
EPS = 1e-6
KAUG = H + 1               # 9 contraction rows (8 heads + aug)
RPT = 4                    # rows per staging tile (one per 32-strip)
N_TILES = R // RPT
NCH = DIN // 128
F32 = mybir.dt.float32
BF16 = mybir.dt.bfloat16
AX = mybir.AluOpType
AF = mybir.ActivationFunctionType
OBW = 124 + 128            # wide-mask width for accumulate-stats matmuls
K_STOP = int(os.environ.get("K_STOP", "9"))

_cache = {}


def _ap_with(ap2d, part_stride_mult, count):
    dims = [list(d) for d in ap2d.ap]
    dims[0] = [dims[0][0] * part_stride_mult, count]
    return bass.AP(tensor=ap2d.tensor, offset=ap2d.offset, ap=dims)


def _bcast(ap_row, count):
    """Partition-broadcast a [1, ...] (DRAM) AP to `count` partitions."""
    dims = [list(d) for d in ap_row.ap]
    assert dims[0][1] == 1, dims
    dims[0] = [0, count]
    return bass.AP(tensor=ap_row.tensor, offset=ap_row.offset, ap=dims)


def host_constants():
    bf = ml_dtypes.bfloat16
    identb = np.eye(128, dtype=np.float32).astype(bf)
    # accumulate masks (bf16): lhsT slice [*, 124-4m : 252-4m] puts row j of
    # tile m (strip j) at output partition 4m+j.
    obw = np.zeros((128, OBW), dtype=np.float32)   # E: sum heads k<8
    pmw = np.zeros((128, OBW), dtype=np.float32)   # Mu: pick aug row k=8
    for p in range(128):
        s, k = p // 32, p % 32
        if k < 8:
            obw[p, 124 + s] = 1.0
        elif k == 8:
            pmw[p, 124 + s] = 1.0
    # block-diag gram mask: keep gsta[p=(s,k), q=(s',m)] iff s==s' and k<8
    bdm = np.zeros((128, 128), dtype=np.float32)
    for p in range(128):
        s, k = p // 32, p % 32
        if k < 8:
            bdm[p, 32 * s:32 * s + 32] = 1.0
    ones_row = np.ones((1, DH), dtype=np.float32).astype(bf)
    return identb, obw.astype(bf), pmw.astype(bf), bdm.astype(bf), ones_row


def build_program():
    if "nc" in _cache:
        return _cache["nc"]
    nc = bacc.Bacc("TRN2", target_bir_lowering=False, debug=False)

    # ---- DRAM I/O ----
    d_xT = nc.dram_tensor("xT", (DIN, NB), F32, kind="ExternalInput")
    d_Wh = nc.dram_tensor("Wh", (DIN, DH), F32, kind="ExternalInput")
    d_bh = nc.dram_tensor("bh", (DH,), F32, kind="ExternalInput")
    d_lns = nc.dram_tensor("ln_scale", (DH,), F32, kind="ExternalInput")
    d_lnb = nc.dram_tensor("ln_bias", (DH,), F32, kind="ExternalInput")
    d_Wz1 = nc.dram_tensor("Wz1", (DIN + A, H * DH), F32, kind="ExternalInput")
    d_Wz2 = nc.dram_tensor("Wz2", (DIN + A, H * DH), F32, kind="ExternalInput")
    d_id = nc.dram_tensor("identb", (128, 128), BF16, kind="ExternalInput")
    d_ob = nc.dram_tensor("obw", (128, OBW), BF16, kind="ExternalInput")
    d_pm = nc.dram_tensor("pmw", (128, OBW), BF16, kind="ExternalInput")
    d_bdm = nc.dram_tensor("bdm", (128, 128), BF16, kind="ExternalInput")
    d_onesr = nc.dram_tensor("ones_row", (1, DH), BF16, kind="ExternalInput")

    d_h = nc.dram_tensor("h_out", (NB, DH), F32, kind="ExternalOutput")
    d_vbf = nc.dram_tensor("vbf_scratch", (NB, DH), BF16)
    d_u1 = nc.dram_tensor("u1_scratch", (NB, H * DH), BF16)
    d_u2 = nc.dram_tensor("u2_scratch", (NB, H * DH), BF16)
    d_invsig = nc.dram_tensor("invsig_scratch", (R, G), BF16)
    d_y = nc.dram_tensor("y_raw", (128, 4 * R), F32, kind="ExternalOutput")

    def _body(tc, ctx):
        consts = ctx.enter_context(tc.tile_pool(name="consts", bufs=1))
        persist = ctx.enter_context(tc.tile_pool(name="persist", bufs=1))
        stg = ctx.enter_context(tc.tile_pool(name="staging", bufs=1))
        wstream = ctx.enter_context(tc.tile_pool(name="wstream", bufs=4))
        statss = ctx.enter_context(tc.tile_pool(name="statss", bufs=2))

        # ---------- constants ----------
        identb = consts.tile([128, 128], BF16)
        nc.sync.dma_start(identb[:], d_id[:])
        obw = consts.tile([128, OBW], BF16)
        nc.sync.dma_start(obw[:], d_ob[:])
        pmw = consts.tile([128, OBW], BF16)
        nc.sync.dma_start(pmw[:], d_pm[:])
        bdm = consts.tile([128, 128], BF16)
        nc.sync.dma_start(bdm[:], d_bdm[:])
        ones_col = consts.tile([128, 1], F32)
        nc.vector.memset(ones_col[:], 1.0)
        eps_col = consts.tile([NB, 1], F32)
        nc.vector.memset(eps_col[:], EPS)

        xTs = consts.tile([128, NCH * NB], F32)
        for i in range(NCH):
            nc.sync.dma_start(xTs[:, i * NB:(i + 1) * NB],
                              d_xT[128 * i:128 * (i + 1), :])
        xTb = consts.tile([128, NCH * NB], BF16)
        nc.vector.tensor_copy(xTb[:], xTs[:])
        whs = []
        for i in range(NCH):
            t = consts.tile([128, DH], F32, name=f"whs_{i}")
            nc.sync.dma_start(t[:], d_Wh[128 * i:128 * (i + 1), :])
            whs.append(t)
        bh_r = consts.tile([NB, DH], F32)
        nc.sync.dma_start(bh_r[:],
                          _bcast(d_bh[:].rearrange("(o d) -> o d", o=1), NB))
        lns_r = consts.tile([NB, DH], F32)
        nc.sync.dma_start(lns_r[:],
                          _bcast(d_lns[:].rearrange("(o d) -> o d", o=1), NB))
        lnb_r = consts.tile([NB, DH], F32)
        nc.sync.dma_start(lnb_r[:],
                          _bcast(d_lnb[:].rearrange("(o d) -> o d", o=1), NB))

        # c tail rows -> bf16 [8a+h, d], then strip-gathered c tiles
        C1f = persist.tile([128, DH], F32)
        C2f = persist.tile([128, DH], F32)
        for a in range(A):
            nc.sync.dma_start(C1f[8 * a:8 * a + 8, :],
                              d_Wz1[DIN + a, :].rearrange("(h d) -> h d", h=H))
            nc.sync.dma_start(C2f[8 * a:8 * a + 8, :],
                              d_Wz2[DIN + a, :].rearrange("(h d) -> h d", h=H))
        C1b = persist.tile([128, DH], BF16)
        nc.vector.tensor_copy(C1b[:], C1f[:])
        C2b = persist.tile([128, DH], BF16)
        nc.vector.tensor_copy(C2b[:], C2f[:])
        NCB = max(1, A // RPT)
        c1gP, c2gP = [], []
        for mb in range(NCB):
            t1 = persist.tile([128, DH], BF16, name=f"c1gP_{mb}")
            t2 = persist.tile([128, DH], BF16, name=f"c2gP_{mb}")
            nc.gpsimd.memset(t1[:], 0.0)
            nc.gpsimd.memset(t2[:], 0.0)
            for j in range(RPT):
                a = RPT * mb + j
                nc.gpsimd.dma_start(t1[32 * j:32 * j + 8, :],
                                    C1b[8 * a:8 * a + 8, :])
                nc.gpsimd.dma_start(t2[32 * j:32 * j + 8, :],
                                    C2b[8 * a:8 * a + 8, :])
                # Z2 aug row carries 1.0 so prod = Z2*zG exposes mu via pmw
                nc.gpsimd.dma_start(t2[32 * j + 8:32 * j + 9, :],
                                    d_onesr[:])
            c1gP.append(t1)
            c2gP.append(t2)

        pstats_cm = tc.tile_pool(name="pstats", bufs=2, space="PSUM")
        pstats = pstats_cm.__enter__()
        pmain = pstats

        # ---------- h path ----------
        ph = pstats.tile([NB, DH], F32, tag="u", bufs=1)
        for i in range(NCH):
            nc.tensor.matmul(ph[:], xTs[:, i * NB:(i + 1) * NB], whs[i][:],
                             start=(i == 0), stop=(i == NCH - 1))
        hpre = persist.tile([NB, DH], F32)
        nc.vector.tensor_add(hpre[:], ph[:], bh_r[:])
        nc.scalar.activation(hpre[:], hpre[:], AF.Relu)
        hstats = persist.tile([NB, 6], F32)
        nc.vector.bn_stats(hstats[:], hpre[:])
        hmv = persist.tile([NB, 2], F32)
        nc.vector.bn_aggr(hmv[:], hstats[:])
        hstd = persist.tile([NB, 1], F32)
        nc.scalar.activation(hstd[:], hmv[:, 1:2], AF.Sqrt, bias=eps_col[:])
        hrstd = persist.tile([NB, 1], F32)
        nc.vector.reciprocal(hrstd[:], hstd[:])
        h_s = persist.tile([NB, DH], F32)
        nc.vector.tensor_scalar(out=h_s[:], in0=hpre[:], scalar1=hmv[:, 0:1],
                                scalar2=hrstd[:], op0=AX.subtract, op1=AX.mult)
        nc.vector.tensor_mul(h_s[:], h_s[:], lns_r[:])
        nc.vector.tensor_add(h_s[:], h_s[:], lnb_r[:])
        nc.sync.dma_start(d_h[:], h_s[:])
        v_bf = persist.tile([NB, DH], BF16)
        nc.vector.tensor_copy(v_bf[:], h_s[:])
        nc.sync.dma_start(d_vbf[:], v_bf[:])
        vrep = persist.tile([128, NB * DH], BF16)
        for b in range(NB):
            nc.sync.dma_start(vrep[:, DH * b:DH * (b + 1)],
                              _bcast(d_vbf[b:b + 1, :], 128))

        if K_STOP < 2:
            return
        # ---------- u1/u2 (bf16 matmuls, wide evac, DRAM roundtrip) --------
        Uw1 = persist.tile([NB, H * DH], BF16)
        Uw2 = persist.tile([NB, H * DH], BF16)
        for dW, Uw, dU in ((d_Wz1, Uw1, d_u1), (d_Wz2, Uw2, d_u2)):
            for hh in range(H):
                pu = pstats.tile([NB, DH], F32, tag="u", name=f"pu_{hh}", bufs=1)
                for i in range(NCH):
                    wz = wstream.tile([128, DH], F32, tag="wz", name="wz")
                    dma_e = nc.sync if (hh + i) % 2 == 0 else nc.gpsimd
                    dma_e.dma_start(
                        wz[:], dW[128 * i:128 * (i + 1), DH * hh:DH * (hh + 1)])
                    wzb = wstream.tile([128, DH], BF16, tag="wzb", name="wzb")
                    nc.vector.tensor_copy(wzb[:], wz[:])
                    nc.tensor.matmul(pu[:], xTb[:, i * NB:(i + 1) * NB],
                                     wzb[:],
                                     start=(i == 0), stop=(i == NCH - 1))
                nc.vector.tensor_copy(Uw[:, DH * hh:DH * (hh + 1)], pu[:])
            nc.sync.dma_start(dU[:], Uw[:])

        if K_STOP < 3:
            return
        # ---------- per-batch u staging + per-tile Z = Zu + c --------------
        Zu1 = [persist.tile([128, DH], BF16, name=f"Zu1_{b}")
               for b in range(NB)]
        Zu2 = [persist.tile([128, DH], BF16, name=f"Zu2_{b}")
               for b in range(NB)]
        for b in range(NB):
            nc.gpsimd.memset(Zu1[b][:], -1.0)
            nc.gpsimd.memset(Zu2[b][:], 0.0)
            for j in range(RPT):
                nc.sync.dma_start(
                    Zu1[b][32 * j:32 * j + 8, :],
                    d_u1[b, :].rearrange("(h d) -> h d", h=H))
                nc.sync.dma_start(
                    Zu2[b][32 * j:32 * j + 8, :],
                    d_u2[b, :].rearrange("(h d) -> h d", h=H))
        Z1 = [stg.tile([128, DH], BF16, tag=f"z1_{m}", name=f"Z1_{m}")
              for m in range(N_TILES)]
        Z2 = [stg.tile([128, DH], BF16, tag=f"z2_{m}", name=f"Z2_{m}")
              for m in range(N_TILES)]
        for m in range(N_TILES):
            bl = (RPT * m) // A
            nc.gpsimd.tensor_add(Z1[m][:], Zu1[bl][:], c1gP[m % NCB][:])
            nc.vector.tensor_add(Z2[m][:], Zu2[bl][:], c2gP[m % NCB][:])
        # transposes of Zu1 / c1gP once; per-tile z1T = sum (linearity)
        Zu1T = []
        c1gPT = []
        for b in range(NB):
            pt = pstats.tile([128, 512], BF16, tag="pw", name=f"ptu_{b}",
                             bufs=2)
            for c in range(4):
                nc.tensor.transpose(pt[:, 128 * c:128 * (c + 1)],
                                    Zu1[b][:, 128 * c:128 * (c + 1)],
                                    identb[:])
            t = persist.tile([128, 512], BF16, name=f"Zu1T_{b}")
            nc.vector.tensor_copy(t[:], pt[:])
            Zu1T.append(t)
        for mb in range(NCB):
            pt = pstats.tile([128, 512], BF16, tag="pw", name=f"ptc_{mb}",
                             bufs=2)
            for c in range(4):
                nc.tensor.transpose(pt[:, 128 * c:128 * (c + 1)],
                                    c1gP[mb][:, 128 * c:128 * (c + 1)],
                                    identb[:])
            t = persist.tile([128, 512], BF16, name=f"c1gPT_{mb}")
            nc.vector.tensor_copy(t[:], pt[:])
            c1gPT.append(t)

        if K_STOP < 4:
            return
        # ---------- quartered stats + main (overlapped by the scheduler) --
        NQ = max(1, R // 32)                    # quarters (32-row aligned)
        NQT = N_TILES // NQ                     # tiles per quarter
        pE = pstats.tile([128, G], F32, tag="pE", bufs=1)
        pMu = pstats.tile([128, G], F32, tag="pMu", bufs=1)
        EAll = persist.tile([128, G], F32)
        MuAll = persist.tile([128, G], F32)
        var = persist.tile([128, G], F32)
        std = persist.tile([128, G], F32)
        invsig = persist.tile([128, G], F32)
        invsigB = persist.tile([128, G], BF16)
        MuBf = persist.tile([128, G], BF16)
        ireps = []
        for i in range(2):
            t = persist.tile([128, G], BF16, name=f"irep_{i}")
            nc.gpsimd.memset(t[:], 0.0)
            ireps.append(t)
        Yall = persist.tile([128, 4 * R], F32)
        qeng = [nc.sync, nc.gpsimd, nc.sync, nc.gpsimd]

        for qq in range(NQ):
            mlo, mhi = qq * NQT, (qq + 1) * NQT
            rlo = mlo * RPT
            nrq = NQT * RPT
            sl32 = slice(rlo, rlo + nrq)
            # --- stats for this quarter's tiles ---
            for m in range(mlo, mhi):
                bl = (RPT * m) // A
                z1T = statss.tile([128, 512], BF16, tag="z1T",
                                  name=f"z1T_{m}")
                nc.vector.tensor_add(z1T[:], Zu1T[bl][:], c1gPT[m % NCB][:])
                pG = pstats.tile([128, 128], F32, tag="u", name=f"pG_{m}",
                                 bufs=1)
                for c in range(4):
                    sc = slice(128 * c, 128 * (c + 1))
                    nc.tensor.matmul(pG[:], z1T[:, sc], z1T[:, sc],
                                     start=(c == 0), stop=(c == 3))
                gsta = statss.tile([128, 128], BF16, tag="gsta",
                                   name=f"gsta_{m}")
                nc.vector.tensor_mul(gsta[:], pG[:], bdm[:])
                pzG = pstats.tile([128, G], F32, tag="pzG", name=f"pzG_{m}",
                                  bufs=1)
                nc.tensor.matmul(pzG[:], gsta[:], Z2[m][:],
                                 start=True, stop=True)
                prod = statss.tile([128, G], BF16, tag="prod",
                                   name=f"prod_{m}")
                nc.vector.tensor_mul(prod[:], Z2[m][:], pzG[:])
                o = 124 - 4 * (m - mlo)
                nc.tensor.matmul(pE[:], obw[:, o:o + 128], prod[:],
                                 start=(m == mlo), stop=(m == mhi - 1),
                                 skip_group_check=True)
                nc.tensor.matmul(pMu[:], pmw[:, o:o + 128], prod[:],
                                 start=(m == mlo), stop=(m == mhi - 1),
                                 skip_group_check=True)
            # --- finalize this quarter ---
            nc.vector.tensor_scalar_mul(EAll[sl32, :], pE[0:nrq, :], 1.0 / G)
            nc.vector.tensor_scalar_mul(MuAll[sl32, :], pMu[0:nrq, :],
                                        -1.0 / G)
            nc.vector.tensor_mul(var[sl32, :], MuAll[sl32, :], MuAll[sl32, :])
            nc.vector.tensor_sub(var[sl32, :], EAll[sl32, :], var[sl32, :])
            nc.scalar.activation(std[sl32, :], var[sl32, :], AF.Sqrt)
            nc.vector.tensor_scalar_add(std[sl32, :], std[sl32, :], EPS)
            nc.vector.reciprocal(invsig[sl32, :], std[sl32, :])
            nc.vector.tensor_copy(invsigB[sl32, :], invsig[sl32, :])
            nc.sync.dma_start(d_invsig[sl32, :], invsigB[sl32, :])
            nc.vector.tensor_copy(MuBf[sl32, :], MuAll[sl32, :])
            # --- scale this quarter's Z2 ---
            for m in range(mlo, mhi):
                irep = ireps[m % 2]
                nc.sync.dma_start(_ap_with(Z2[m][8:9, :], 32, 4),
                                  MuBf[RPT * m:RPT * m + RPT, :])
                for j in range(RPT):
                    r = RPT * m + j
                    qeng[j].dma_start(irep[32 * j:32 * j + 9, :],
                                      _bcast(d_invsig[r:r + 1, :], 9))
                nc.vector.tensor_mul(Z2[m][:], Z2[m][:], irep[:])
            if K_STOP < 7:
                continue
            # --- main rows of this quarter ---
            for m in range(mlo, mhi):
                for j in range(RPT):
                    r = RPT * m + j
                    bl = r // A
                    p0 = 32 * j
                    ts_ = statss.tile([128, 4 * G], BF16, tag="tanh",
                                      name=f"t_{r}")
                    for half in range(2):
                        pw = pstats.tile([128, 2 * G], F32, tag="pw",
                                         name=f"pw_{r}_{half}")
                        for cc in range(2):
                            c = 2 * half + cc
                            nc.tensor.matmul(
                                pw[:, G * cc:G * (cc + 1)],
                                Z2[m][p0:p0 + KAUG, 128 * c:128 * (c + 1)],
                                Z1[m][p0:p0 + KAUG, :],
                                start=True, stop=True,
                                tile_position=(p0, 0))
                        nc.scalar.activation(
                            ts_[:, 2 * G * half:2 * G * (half + 1)], pw[:],
                            AF.Tanh)
                    if K_STOP < 8:
                        continue
                    tdump = statss.tile([128, G], BF16, tag="tdump",
                                        name=f"td_{r}")
                    for c in range(4):
                        nc.vector.scalar_tensor_tensor(
                            out=tdump[:], in0=ts_[:, G * c:G * (c + 1)],
                            scalar=1.0, in1=vrep[:, DH * bl:DH * (bl + 1)],
                            op0=AX.mult, op1=AX.mult,
                            accum_out=Yall[:, 4 * r + c:4 * r + c + 1])
        if K_STOP < 9:
            return
        nc.sync.dma_start(d_y[:], Yall[:])
        pstats_cm.__exit__(None, None, None)

    with tile.TileContext(nc) as tc, ExitStack() as ctx:
        _body(tc, ctx)
    nc.compile()
    _cache["nc"] = nc
    return nc


def kernel(x, Wh, bh, ln_scale, ln_bias, Wz1, Wz2):
    x = np.ascontiguousarray(np.asarray(x, dtype=np.float32))
    Wh = np.ascontiguousarray(np.asarray(Wh, dtype=np.float32))
    bh = np.ascontiguousarray(np.asarray(bh, dtype=np.float32))
    ln_scale = np.ascontiguousarray(np.asarray(ln_scale, dtype=np.float32))
    ln_bias = np.ascontiguousarray(np.asarray(ln_bias, dtype=np.float32))
    Wz1 = np.ascontiguousarray(np.asarray(Wz1, dtype=np.float32))
    Wz2 = np.ascontiguousarray(np.asarray(Wz2, dtype=np.float32))

    nc = build_program()
    identb, obw, pmw, bdm, ones_row = host_constants()
    in_maps = []
    for c in range(N_CORES):
        xT = np.ascontiguousarray(x[NB * c:NB * (c + 1), :].T)
        in_maps.append({
            "xT": xT, "Wh": Wh, "bh": bh, "ln_scale": ln_scale,
            "ln_bias": ln_bias, "Wz1": Wz1, "Wz2": Wz2,
            "identb": identb, "obw": obw, "pmw": pmw, "bdm": bdm,
            "ones_row": ones_row,
        })
    res = run_bass_kernel_spmd(nc, in_maps, core_ids=list(range(N_CORES)),
                               trace=bool(int(os.environ.get("K_TRACE", "0"))))
    if res.exec_time_ns is not None:
        _cache["exec_time_ns"] = res.exec_time_ns
        _cache["trace"] = res.instructions_and_trace
    h = np.concatenate([r["h_out"] for r in res.results], axis=0)
    ys = []
    for r_ in res.results:
        yr = r_["y_raw"].reshape(128, R, 4)       # p, r, c
        ys.append(np.transpose(yr, (1, 2, 0)).reshape(R, G))
    y_ = np.concatenate(ys, axis=0)
    q = (y_.astype(np.float64) ** 2).mean(axis=1).astype(np.float32)
    q = q.reshape(B, A) if NB * N_CORES == B else q.reshape(-1, A)
    return h, q, y_
